# revision 1
# baseline (speedup 1.0000x reference)
"""DelayRNN Trainium2 kernel.

Sharding (hardcoded from spec): data-parallel over batch. B=32 rows are
sharded 4-per-core across 8 NeuronCores; every core holds all weights in
SBUF and runs the full 256-step encode + 64-step decode recurrence for its
4 rows. No cross-core communication.

Math reformulation (validated vs reference to ~3e-6):
  Wh = W_in[:H], Wx = W_in[H:]
  Wp2 = Wh @ W_pass                       (folded pass matrix, on device)
  cx_t = x_t @ Wx + b_in ;  cp_t = x_t @ (Wx@W_pass) + (b_in@W_pass+b_pass)
  cmix_t = m_t ? cp_t : cx_t              (precomputed batched, kept in DRAM)
  h'_t = (1-m)*h0@Wh + m*h0@Wp2 + cmix_t  (PSUM-accumulated; mask folded
                                           into pre-scaled stationaries)
  tau' = max(16*sigmoid(h'@W_tau + b_tau), 1) ; mem = sigmoid(h'@W_mem+b_mem)
  buffer slots 1..16 (idx 0..15): h0_next = buf[0] + (mem*h')/tau'
  nbuf[0:15] = buf[1:16] + q*r[1:16], q = mem*h', r_d = 1/(1+|tau'-d|)
Decode: h = h0@Wp2 + cdec; out_t = h0@W_out + b_out batched at the end.

All matmul streams run as float32r (1 cyc/row at N=512 vs 4 for fp32); the
BIR verifier requires fp32r matmul inputs to be *produced* rounded, so
weights get a one-time rounding copy and recurrent stationaries are written
as fp32r by their producing engine ops.
"""

import sys
import numpy as np

for _p in ("/opt/trn_rl_repo",):
    if _p not in sys.path:
        sys.path.append(_p)

from contextlib import ExitStack

import concourse.bass as bass
import concourse.tile as tile
from concourse import bacc, mybir
from concourse.masks import make_identity

FP32 = mybir.dt.float32
FP32R = mybir.dt.float32r
BF16 = mybir.dt.bfloat16
I32 = mybir.dt.int32

B, S, I, H, C = 32, 256, 128, 512, 64
T_OUT = 64
NCORES = 8
BL = B // NCORES        # 4 batch rows per core
KC = H // 128           # 4 k-chunks
D = 16                  # delay slots 1..16
CB = KC * BL            # 16 = chunks x batch (free size of ^T tiles)

Sig = mybir.ActivationFunctionType.Sigmoid
Op = mybir.AluOpType


def f32(ap):
    return ap.bitcast(FP32)


def build(seq_len=S, t_out=T_OUT, zero_bias=True):
    nc = bacc.Bacc("TRN2", target_bir_lowering=False, debug=False)

    # ---------------- DRAM I/O ----------------
    dx = nc.dram_tensor("x", [BL, seq_len, I], FP32, kind="ExternalInput")
    dlen = nc.dram_tensor("lengths", [BL], I32, kind="ExternalInput")
    dwin = nc.dram_tensor("W_in", [I + H, H], FP32, kind="ExternalInput")
    dwpass = nc.dram_tensor("W_pass", [H, H], FP32, kind="ExternalInput")
    dwtau = nc.dram_tensor("W_tau", [H, H], FP32, kind="ExternalInput")
    dwmem = nc.dram_tensor("W_mem", [H, H], FP32, kind="ExternalInput")
    dwout = nc.dram_tensor("W_out", [H, C], FP32, kind="ExternalInput")
    dbias = {}
    for nm, ln in [("b_in", H), ("b_pass", H), ("b_tau", H),
                   ("b_mem", H), ("b_out", C)]:
        dbias[nm] = nc.dram_tensor(nm, [ln], FP32, kind="ExternalInput")
    dout = nc.dram_tensor("out", [BL, t_out, C], FP32, kind="ExternalOutput")
    # internal DRAM scratch (fp32r so the per-step reload is pre-rounded)
    dcmix = nc.dram_tensor("cmix_scratch", [BL * seq_len, H], FP32R)

    NROW = BL * seq_len            # bt rows
    NMT = NROW // 128              # row tiles for the cx/cp precompute
    TPB = seq_len // 128           # row tiles per batch row

    with tile.TileContext(nc) as tc, ExitStack() as ctx:
        persist = ctx.enter_context(tc.tile_pool(name="persist", bufs=1))

        # ------------- persistent SBUF tensors -------------
        wh = persist.tile([128, KC, H], FP32R, name="wh")     # stream [kp,kc,n]
        wp2 = persist.tile([128, KC, H], FP32R, name="wp2")
        wtau = persist.tile([128, KC, H], FP32R, name="wtau")
        wmem = persist.tile([128, KC, H], FP32R, name="wmem")
        wout = persist.tile([128, KC, C], FP32R, name="wout")
        i4r = persist.tile([4, 4], FP32R, name="i4r")
        iota16 = persist.tile([128, D], FP32, name="iota16")  # 1..16, all parts
        maskR = persist.tile([128, seq_len, BL], FP32, name="maskR")
        buf0 = persist.tile([128, CB, D], FP32, name="buf0")
        buf1 = persist.tile([128, CB, D], FP32, name="buf1")
        h0coll = persist.tile([128, KC, t_out, BL], FP32R, name="h0coll")
        if not zero_bias:
            btauT = persist.tile([128, KC], FP32, name="btauT")
            bmemT = persist.tile([128, KC], FP32, name="bmemT")
            b_out_r = persist.tile([128, C], FP32, name="b_out_r")
            cdec4 = persist.tile([BL, H], FP32R, name="cdec4")

        # ------------- setup (scoped pools) -------------
        with tc.tile_pool(name="setup_ps", bufs=2, space="PSUM") as setup_ps, \
                tc.tile_pool(name="setup_sb", bufs=2) as setup_sb:
            # raw fp32 weight loads
            wh_d = setup_sb.tile([128, KC, H], FP32, name="wh_d")
            wpass_d = setup_sb.tile([128, KC, H], FP32, name="wpass_d")
            wtau_d = setup_sb.tile([128, KC, H], FP32, name="wtau_d")
            wmem_d = setup_sb.tile([128, KC, H], FP32, name="wmem_d")
            wx_d = setup_sb.tile([128, H], FP32, name="wx_d")
            wout_d = setup_sb.tile([128, KC, C], FP32, name="wout_d")
            nc.sync.dma_start(wh_d[:], dwin[:H].rearrange(
                "(kc kp) n -> kp kc n", kp=128))
            nc.sync.dma_start(wx_d[:], dwin[H:])
            nc.sync.dma_start(wpass_d[:], dwpass[:].rearrange(
                "(kc kp) n -> kp kc n", kp=128))
            nc.sync.dma_start(wtau_d[:], dwtau[:].rearrange(
                "(kc kp) n -> kp kc n", kp=128))
            nc.sync.dma_start(wmem_d[:], dwmem[:].rearrange(
                "(kc kp) n -> kp kc n", kp=128))
            nc.sync.dma_start(wout_d[:], dwout[:].rearrange(
                "(kc kp) n -> kp kc n", kp=128))
            # rounding copies into fp32r
            wpass_r = setup_sb.tile([128, KC, H], FP32R, name="wpass_r")
            wx_r = setup_sb.tile([128, H], FP32R, name="wx_r")
            nc.vector.tensor_copy(wh[:], wh_d[:])
            nc.vector.tensor_copy(wtau[:], wtau_d[:])
            nc.vector.tensor_copy(wmem[:], wmem_d[:])
            nc.vector.tensor_copy(wpass_r[:], wpass_d[:])
            nc.vector.tensor_copy(wx_r[:], wx_d[:])
            nc.vector.tensor_copy(wout[:], wout_d[:])

            i4f = setup_sb.tile([4, 4], FP32, name="i4f")
            make_identity(nc, i4f[:])
            nc.vector.tensor_copy(i4r[:], i4f[:])
            id128 = setup_sb.tile([128, 128], FP32, name="id128")
            make_identity(nc, id128[:])

            iota16_i = setup_sb.tile([128, D], I32, name="iota16_i")
            nc.gpsimd.iota(iota16_i[:], pattern=[[1, D]], base=1,
                           channel_multiplier=0)
            nc.vector.tensor_copy(iota16[:], iota16_i[:])

            # masks, replicated on every partition: maskR[p, t, b] = t < len[b]
            iota_t = setup_sb.tile([128, seq_len], I32, name="iota_t")
            nc.gpsimd.iota(iota_t[:], pattern=[[1, seq_len]], base=0,
                           channel_multiplier=0)
            lenR = setup_sb.tile([128, BL], I32, name="lenR")
            nc.sync.dma_start(
                lenR[:], dlen[:].unsqueeze(0).to_broadcast([128, BL]))
            mkR_i = setup_sb.tile([128, seq_len, BL], I32, name="mkR_i")
            nc.vector.tensor_tensor(
                out=mkR_i[:],
                in0=iota_t[:].unsqueeze(2).to_broadcast([128, seq_len, BL]),
                in1=lenR[:].unsqueeze(1).to_broadcast([128, seq_len, BL]),
                op=Op.is_lt)
            nc.vector.tensor_copy(maskR[:], mkR_i[:])

            # mask_bt[p, m], m = b*TPB + j, row r = 128*m + p (int mask
            # for copy_predicated)
            iota_bt = setup_sb.tile([128, TPB], I32, name="iota_bt")
            nc.gpsimd.iota(iota_bt[:], pattern=[[128, TPB]], base=0,
                           channel_multiplier=1)
            mk_bt_i = setup_sb.tile([128, BL, TPB], I32, name="mk_bt_i")
            nc.vector.tensor_tensor(
                out=mk_bt_i[:],
                in0=iota_bt[:].unsqueeze(1).to_broadcast([128, BL, TPB]),
                in1=lenR[:].unsqueeze(2).to_broadcast([128, BL, TPB]),
                op=Op.is_lt)

            # WhT / WxT via PE transposes (fp32 path), rounded to fp32r
            whT = setup_sb.tile([128, KC, H], FP32R, name="whT")
            wxT = setup_sb.tile([128, KC, I], FP32R, name="wxT")
            for jc in range(KC):
                for kc in range(KC):
                    pst = setup_ps.tile([128, 128], FP32, tag="setup_T")
                    nc.tensor.transpose(pst[:], wh_d[:, kc, bass.ts(jc, 128)],
                                        id128[:])
                    nc.vector.tensor_copy(whT[:, jc, bass.ts(kc, 128)],
                                          pst[:])
            for jc in range(KC):
                pst = setup_ps.tile([128, 128], FP32, tag="setup_T")
                nc.tensor.transpose(pst[:], wx_d[:, bass.ts(jc, 128)],
                                    id128[:])
                nc.vector.tensor_copy(wxT[:, jc, :], pst[:])

            # Wp2 = Wh @ W_pass ; Wxp = Wx @ W_pass
            wxp = setup_sb.tile([128, H], FP32R, name="wxp")
            for m in range(KC):
                psg = setup_ps.tile([128, H], FP32, tag="setup_G")
                for jc in range(KC):
                    nc.tensor.matmul(psg[:], whT[:, jc, bass.ts(m, 128)],
                                     wpass_r[:, jc, :],
                                     start=(jc == 0), stop=(jc == KC - 1))
                nc.vector.tensor_copy(wp2[:, m, :], psg[:])
            psg = setup_ps.tile([128, H], FP32, tag="setup_G")
            for jc in range(KC):
                nc.tensor.matmul(psg[:], wxT[:, jc, :], wpass_r[:, jc, :],
                                 start=(jc == 0), stop=(jc == KC - 1))
            nc.vector.tensor_copy(wxp[:], psg[:])

            # decode constant cdec = b_in @ W_pass + b_pass
            if not zero_bias:
                nc.sync.dma_start(btauT[:], dbias["b_tau"][:].rearrange(
                    "(c p) -> p c", p=128))
                nc.sync.dma_start(bmemT[:], dbias["b_mem"][:].rearrange(
                    "(c p) -> p c", p=128))
                nc.sync.dma_start(
                    b_out_r[:], dbias["b_out"][:].unsqueeze(0)
                    .to_broadcast([128, C]))
                b_in_r = setup_sb.tile([128, H], FP32, name="b_in_r")
                nc.sync.dma_start(
                    b_in_r[:], dbias["b_in"][:].unsqueeze(0)
                    .to_broadcast([128, H]))
                binT4 = setup_sb.tile([128, KC, BL], FP32R, name="binT4")
                binT4_d = setup_sb.tile([128, KC, BL], FP32, name="binT4_d")
                nc.sync.dma_start(
                    binT4_d[:],
                    dbias["b_in"][:].rearrange("(c p) -> p c", p=128)
                    .unsqueeze(2).to_broadcast([128, KC, BL]))
                nc.vector.tensor_copy(binT4[:], binT4_d[:])
                psd = setup_ps.tile([BL, H], FP32, tag="setup_D")
                for c in range(KC):
                    nc.tensor.matmul(psd[:], binT4[:, c, :], wpass_r[:, c, :],
                                     start=(c == 0), stop=(c == KC - 1))
                bps = setup_sb.tile([BL, H], FP32, name="bps")
                nc.sync.dma_start(
                    bps[:], dbias["b_pass"][:].unsqueeze(0)
                    .to_broadcast([BL, H]))
                nc.vector.tensor_tensor(out=cdec4[:], in0=psd[:],
                                        in1=bps[:], op=Op.add)
                dcdec = nc.dram_tensor("cdec_scratch", [H], FP32)
                cdec_r = setup_sb.tile([128, H], FP32, name="cdec_r")
                nc.sync.dma_start(dcdec[:], f32(cdec4[0:1, :]).squeeze(0))
                nc.sync.dma_start(
                    cdec_r[:], dcdec[:].unsqueeze(0).to_broadcast([128, H]))

            # x -> xT ; cx/cp/cmix precompute
            x_sb = setup_sb.tile([128, NMT, I], FP32, name="x_sb")
            xT = setup_sb.tile([128, NMT, 128], FP32R, name="xT")
            nc.sync.dma_start(
                x_sb[:],
                dx[:].rearrange("b t i -> (b t) i").rearrange(
                    "(m p) i -> p m i", p=128))
            for m in range(NMT):
                pst = setup_ps.tile([128, 128], FP32, tag="setup_T")
                nc.tensor.transpose(pst[:], x_sb[:, m, :], id128[:])
                nc.vector.tensor_copy(xT[:, m, :], pst[:])
            for m in range(NMT):
                ps1 = setup_ps.tile([128, H], FP32, tag="setup_G")
                nc.tensor.matmul(ps1[:], xT[:, m, :], wx_r[:],
                                 start=True, stop=True)
                cxt = setup_sb.tile([128, H], FP32R, tag="cxt", bufs=3)
                if zero_bias:
                    nc.vector.tensor_copy(cxt[:], ps1[:])
                else:
                    nc.vector.tensor_tensor(out=cxt[:], in0=ps1[:],
                                            in1=b_in_r[:], op=Op.add)
                ps2 = setup_ps.tile([128, H], FP32, tag="setup_G")
                nc.tensor.matmul(ps2[:], xT[:, m, :], wxp[:],
                                 start=True, stop=True)
                cpt = setup_sb.tile([128, H], FP32R, tag="cpt", bufs=3)
                if zero_bias:
                    nc.vector.tensor_copy(cpt[:], ps2[:])
                else:
                    nc.vector.tensor_tensor(out=cpt[:], in0=ps2[:],
                                            in1=cdec_r[:], op=Op.add)
                nc.vector.copy_predicated(
                    out=f32(cxt[:]),
                    mask=mk_bt_i[:, m // TPB, m % TPB].unsqueeze(1)
                    .to_broadcast([128, H]),
                    data=f32(cpt[:]))
                nc.sync.dma_start(dcmix[bass.ts(m, 128), :], cxt[:])

        # ------------- main recurrence -------------
        psum = ctx.enter_context(tc.tile_pool(name="mn_ps", bufs=1,
                                              space="PSUM"))
        loop_sb = ctx.enter_context(tc.tile_pool(name="mn_sb", bufs=2))
        dma_sb = ctx.enter_context(tc.tile_pool(name="mn_dma", bufs=4))

        cmix_v = dcmix[:].rearrange("(b t) n -> b t n", b=BL)
        bufs = [buf0, buf1]

        h0 = loop_sb.tile([128, CB], FP32R, tag="h0")
        nc.vector.memset(f32(h0[:]), 0.0)
        nc.gpsimd.memset(buf0[:], 0.0)

        deferred = []

        buf_idx = 0
        total_steps = seq_len + t_out
        for t in range(total_steps):
            is_enc = t < seq_len
            td = t - seq_len
            last = (t == total_steps - 1)

            if not is_enc:
                nc.vector.tensor_copy(
                    h0coll[:, :, td, :],
                    f32(h0[:]).rearrange("p (c b) -> p c b", c=KC))
                if last:
                    break

            # critical stationaries for L1
            if is_enc:
                mT = maskR[:, t, :].unsqueeze(1).to_broadcast([128, KC, BL])
                lhs_b = loop_sb.tile([128, CB], FP32R, tag="lhs_b")
                nc.vector.tensor_tensor(
                    out=lhs_b[:].rearrange("p (c b) -> p c b", c=KC),
                    in0=f32(h0[:]).rearrange("p (c b) -> p c b", c=KC),
                    in1=mT, op=Op.mult)
                lhs_a = loop_sb.tile([128, CB], FP32R, tag="lhs_a")
                nc.vector.tensor_tensor(out=lhs_a[:], in0=f32(h0[:]),
                                        in1=f32(lhs_b[:]), op=Op.subtract)

            # deferred buffer update from the previous step
            while deferred:
                deferred.pop(0)()

            # L1 matmuls
            ps_h = psum.tile([BL, H], FP32, tag="ps_h")
            if is_enc:
                cmix4 = dma_sb.tile([BL, H], FP32R, tag="cmix4")
                nc.sync.dma_start(cmix4[:], cmix_v[:, t, :])
                lhs_a_v = lhs_a[:].rearrange("p (c b) -> p c b", c=KC)
                lhs_b_v = lhs_b[:].rearrange("p (c b) -> p c b", c=KC)
                for c in range(KC):
                    nc.tensor.matmul(ps_h[:], lhs_a_v[:, c, :], wh[:, c, :],
                                     start=(c == 0), stop=False)
                for c in range(KC):
                    nc.tensor.matmul(ps_h[:], lhs_b_v[:, c, :], wp2[:, c, :],
                                     start=False, stop=(c == KC - 1))

            else:
                h0_v = h0[:].rearrange("p (c b) -> p c b", c=KC)
                for c in range(KC):
                    nc.tensor.matmul(ps_h[:], h0_v[:, c, :], wp2[:, c, :],
                                     start=(c == 0), stop=(c == KC - 1))

            # T1: h' -> h'^T  (h' = psum + cmix, folded into the copy)
            h_sb = loop_sb.tile([BL, H], FP32R, tag="h_sb")
            if is_enc:
                nc.vector.tensor_tensor(out=h_sb[:], in0=ps_h[:],
                                        in1=f32(cmix4[:]), op=Op.add)
            elif not zero_bias:
                nc.vector.tensor_tensor(out=h_sb[:], in0=ps_h[:],
                                        in1=f32(cdec4[:]), op=Op.add)
            else:
                nc.scalar.copy(h_sb[:], ps_h[:])
            ps_t1 = psum.tile([128, KC, BL], FP32, tag="ps_T1")
            for c in range(KC):
                nc.tensor.matmul(ps_t1[:, c, :], h_sb[:, bass.ts(c, 128)],
                                 i4r[:], start=True, stop=True)
            hT = loop_sb.tile([128, CB], FP32R, tag="hT")
            nc.vector.tensor_copy(
                hT[:].rearrange("p (c b) -> p c b", c=KC), ps_t1[:])

            # L2 matmuls
            hT_v = hT[:].rearrange("p (c b) -> p c b", c=KC)
            ps_tau = psum.tile([BL, H], FP32, tag="ps_tau")
            ps_mem = psum.tile([BL, H], FP32, tag="ps_mem")
            for c in range(KC):
                nc.tensor.matmul(ps_tau[:], hT_v[:, c, :], wtau[:, c, :],
                                 start=(c == 0), stop=(c == KC - 1))
            for c in range(KC):
                nc.tensor.matmul(ps_mem[:], hT_v[:, c, :], wmem[:, c, :],
                                 start=(c == 0), stop=(c == KC - 1))

            # T2: tau_lin, mem_lin -> ^T (fp32r identity matmuls)
            tau_r = loop_sb.tile([BL, H], FP32R, tag="tau_r")
            nc.scalar.copy(tau_r[:], ps_tau[:])
            mem_r = loop_sb.tile([BL, H], FP32R, tag="mem_r")
            nc.vector.tensor_copy(mem_r[:], ps_mem[:])
            ps_t2 = psum.tile([128, 2, KC, BL], FP32, tag="ps_T2")
            for c in range(KC):
                nc.tensor.matmul(ps_t2[:, 0, c, :],
                                 tau_r[:, bass.ts(c, 128)], i4r[:],
                                 start=True, stop=True)
            for c in range(KC):
                nc.tensor.matmul(ps_t2[:, 1, c, :],
                                 mem_r[:, bass.ts(c, 128)], i4r[:],
                                 start=True, stop=True)
            tmT = loop_sb.tile([128, 2, CB], FP32, tag="tmT")
            nc.vector.tensor_copy(
                tmT[:].rearrange("p a (c b) -> p a c b", c=KC), ps_t2[:])
            if not zero_bias:
                for j, bt in ((0, btauT), (1, bmemT)):
                    nc.vector.tensor_tensor(
                        out=tmT[:, j].rearrange("p (c b) -> p c b", c=KC),
                        in0=tmT[:, j].rearrange("p (c b) -> p c b", c=KC),
                        in1=bt[:].unsqueeze(2).to_broadcast([128, KC, BL]),
                        op=Op.add)

            # sigmoid + critical h0 update
            sig = loop_sb.tile([128, 2, CB], FP32, tag="sig")
            nc.scalar.activation(sig[:], tmT[:], Sig)
            taup = loop_sb.tile([128, CB], FP32, tag="taup")
            nc.vector.tensor_scalar(out=taup[:], in0=sig[:, 0], scalar1=16.0,
                                    scalar2=1.0, op0=Op.mult, op1=Op.max)
            q = loop_sb.tile([128, CB], FP32, tag="q")
            nc.vector.tensor_tensor(out=q[:], in0=sig[:, 1], in1=f32(hT[:]),
                                    op=Op.mult)
            rtau = loop_sb.tile([128, CB], FP32, tag="rtau")
            nc.vector.reciprocal(out=rtau[:], in_=taup[:])
            t1 = loop_sb.tile([128, CB], FP32, tag="t1")
            nc.vector.tensor_tensor(out=t1[:], in0=q[:], in1=rtau[:],
                                    op=Op.mult)
            bcur, bnxt = bufs[buf_idx], bufs[buf_idx ^ 1]
            buf_idx ^= 1
            h0 = loop_sb.tile([128, CB], FP32R, tag="h0")
            nc.vector.tensor_tensor(out=h0[:], in0=bcur[:, :, 0], in1=t1[:],
                                    op=Op.add)

            # deferred: big W + buffer shift-add (runs during next step)
            if t < total_steps - 2:
                def make_deferred(taup=taup, q=q, bcur=bcur, bnxt=bnxt):
                    def emit():
                        wt = loop_sb.tile([128, CB, D], FP32, tag="wt")
                        nc.vector.tensor_tensor(
                            out=wt[:],
                            in0=taup[:].unsqueeze(2).to_broadcast(
                                [128, CB, D]),
                            in1=iota16[:].unsqueeze(1).to_broadcast(
                                [128, CB, D]),
                            op=Op.subtract)
                        nc.scalar.activation(
                            wt[:], wt[:], mybir.ActivationFunctionType.Abs)
                        nc.vector.tensor_scalar(
                            out=wt[:], in0=wt[:], scalar1=1.0, scalar2=None,
                            op0=Op.add)
                        wr = loop_sb.tile([128, CB, D], FP32, tag="wr")
                        ws = loop_sb.tile([128, CB, D], FP32, tag="ws")
                        nc.vector.reciprocal_approx_accurate(
                            out=wr[:], in_=wt[:], scratch=ws[:])
                        nc.vector.tensor_tensor(
                            out=wr[:], in0=wr[:],
                            in1=q[:].unsqueeze(2).to_broadcast([128, CB, D]),
                            op=Op.mult)
                        nc.gpsimd.tensor_tensor(
                            out=bnxt[:, :, 0:D - 1], in0=bcur[:, :, 1:D],
                            in1=wr[:, :, 1:D], op=Op.add)
                        nc.gpsimd.memset(bnxt[:, :, D - 1], 0.0)
                    return emit
                deferred.append(make_deferred())

        # batched output GEMM: out = h0coll @ W_out + b_out
        rows = t_out * BL
        dout_tb = dout[:].transpose([1, 0, 2])      # [t, b, c]
        for start in range(0, rows, 128):
            mrows = min(128, rows - start)
            t0, tn = start // BL, mrows // BL
            ps_o = psum.tile([mrows, C], FP32, tag="ps_o")
            for c in range(KC):
                nc.tensor.matmul(
                    ps_o[:], h0coll[:, c, t0:t0 + tn, :], wout[:, c, :],
                    start=(c == 0), stop=(c == KC - 1))
            out_sb = loop_sb.tile([mrows, C], FP32, tag="out_sb")
            if zero_bias:
                nc.vector.tensor_copy(out_sb[:], ps_o[:])
            else:
                nc.vector.tensor_tensor(out=out_sb[:], in0=ps_o[:],
                                        in1=b_out_r[0:mrows, :], op=Op.add)
            for tt in range(tn):
                nc.sync.dma_start(dout[:, t0 + tt, :],
                                  out_sb[bass.ts(tt, BL), :])

    nc.compile()
    return nc


_CACHE = {}


def _get_module(seq_len, t_out, zero_bias):
    key = (seq_len, t_out, zero_bias)
    if key not in _CACHE:
        _CACHE[key] = build(seq_len, t_out, zero_bias)
    return _CACHE[key]


def kernel(**inputs):
    x = np.ascontiguousarray(np.asarray(inputs["x"], dtype=np.float32))
    lengths = np.ascontiguousarray(
        np.asarray(inputs["lengths"]).astype(np.int32))
    t_out = int(inputs["out_lengths"])
    seq_len = x.shape[1]
    names = ["W_in", "W_pass", "W_tau", "W_mem", "W_out",
             "b_in", "b_pass", "b_tau", "b_mem", "b_out"]
    warrs = {n: np.ascontiguousarray(np.asarray(inputs[n], dtype=np.float32))
             for n in names}
    zero_bias = all(not np.any(warrs[n]) for n in
                    ["b_in", "b_pass", "b_tau", "b_mem", "b_out"])
    nc = _get_module(seq_len, t_out, zero_bias)

    from concourse import bass_utils
    in_maps = []
    for c in range(NCORES):
        sl = slice(c * BL, (c + 1) * BL)
        m = {"x": x[sl], "lengths": lengths[sl]}
        m.update(warrs)
        in_maps.append(m)
    res = bass_utils.run_bass_kernel_spmd(
        nc, in_maps, core_ids=list(range(NCORES)))
    out = np.concatenate([res.results[c]["out"] for c in range(NCORES)],
                         axis=0)
    return out



# revision 2
# speedup vs baseline: 1.1556x; 1.1556x over previous
"""DelayRNN Trainium2 kernel, v2: bf16 row-form recurrence.

Sharding: data-parallel over batch, 4 rows/core on 8 cores (as v1).

Changes vs v1 (fp32r, 4.45 ms):
  * all matmul operands bf16 — moving weight streams run ~5x faster than
    the fp32r ones measured on HW (fp32 LDWEIGHTS + fp32_mode matmuls
    dominated v1), transposes ~3.5x faster
  * cmix (x-dependent injection with the binary length mask folded in)
    lives in SBUF transposed ([128,(kc,t,b)] bf16) and is added during
    the psum->sbuf hop after T1 — no per-step DRAM DMA, no [4,512] add
  * tau/mem copies merge into one [8,512] tile so T2 is 4 transposes
    (not 8); sigmoid reads the T2 PSUM directly (ACT psum port)
  * L2 tau/mem chains are col-tiled to distinct 32-col strips of the PE
    array (tile_position), streaming concurrently on separate XBUSes
  * 1/tau via reciprocal_approx_accurate (2 cheap DVE ops)

Per-step chain: DVE(lb) -> PE(L1 8mm) -> DVE+ACT(bridge copy halves) ->
PE(T1 x4) -> DVE(hT = T1 + cmixT_t) -> PE(L2 2x4mm col-tiled) ->
ACT+DVE(t8 copy halves) -> PE(T2 x4 merged) -> ACT(sigmoid from psum)
-> DVE(taup, q, 1/taup, t1, g'). Buffer slots 2..16 stay deferred one
step and run on DVE/ACT/Pool underneath the next step's PE blocks.
"""

import sys
import numpy as np

for _p in ("/opt/trn_rl_repo",):
    if _p not in sys.path:
        sys.path.append(_p)

from contextlib import ExitStack

import concourse.bass as bass
import concourse.tile as tile
from concourse import bacc, mybir
from concourse.masks import make_identity

FP32 = mybir.dt.float32
FP32R = mybir.dt.float32r
BF16 = mybir.dt.bfloat16
I32 = mybir.dt.int32

B, S, I, H, C = 32, 256, 128, 512, 64
T_OUT = 64
NCORES = 8
BL = B // NCORES        # 4 batch rows per core
KC = H // 128           # 4 k-chunks
D = 16                  # delay slots 1..16
CB = KC * BL            # 16 = chunks x batch

Sig = mybir.ActivationFunctionType.Sigmoid
Abs = mybir.ActivationFunctionType.Abs
Op = mybir.AluOpType

COL_TILE_L2 = True      # run tau/mem moving streams on separate col strips
# keep-warm filler matmuls per PE gap (HAM re-throttles the PE clock to
# 1.2 GHz when it sees idle windows; these keep it at 2.4 GHz)
FILL_L1, FILL_L2, FILL_T2 = 4, 6, 12


def f32(ap):
    return ap.bitcast(FP32)


def build(seq_len=S, t_out=T_OUT, zero_bias=True):
    nc = bacc.Bacc("TRN2", target_bir_lowering=False, debug=False)

    # ---------------- DRAM I/O ----------------
    dx = nc.dram_tensor("x", [BL, seq_len, I], FP32, kind="ExternalInput")
    dlen = nc.dram_tensor("lengths", [BL], I32, kind="ExternalInput")
    dwin = nc.dram_tensor("W_in", [I + H, H], FP32, kind="ExternalInput")
    dwpass = nc.dram_tensor("W_pass", [H, H], FP32, kind="ExternalInput")
    dwtau = nc.dram_tensor("W_tau", [H, H], FP32, kind="ExternalInput")
    dwmem = nc.dram_tensor("W_mem", [H, H], FP32, kind="ExternalInput")
    dwout = nc.dram_tensor("W_out", [H, C], FP32, kind="ExternalInput")
    dbias = {}
    for nm, ln in [("b_in", H), ("b_pass", H), ("b_tau", H),
                   ("b_mem", H), ("b_out", C)]:
        dbias[nm] = nc.dram_tensor(nm, [ln], FP32, kind="ExternalInput")
    dout = nc.dram_tensor("out", [BL, t_out, C], FP32, kind="ExternalOutput")

    NROW = BL * seq_len            # bt rows
    NMT = NROW // 128              # row tiles for the cx/cp precompute
    TPB = seq_len // 128           # row tiles per batch row

    with tile.TileContext(nc) as tc, ExitStack() as ctx:
        persist = ctx.enter_context(tc.tile_pool(name="persist", bufs=1))

        # ------------- persistent SBUF tensors -------------
        whb = persist.tile([128, KC, H], BF16, name="whb")    # moving [kp,c,n]
        wdb = persist.tile([128, KC, H], BF16, name="wdb")    # Wp2 - Wh
        wp2b = persist.tile([128, KC, H], BF16, name="wp2b")
        wtaub = persist.tile([128, KC, H], BF16, name="wtaub")
        wmemb = persist.tile([128, KC, H], BF16, name="wmemb")
        woutb = persist.tile([128, KC, C], BF16, name="woutb")
        i4b = persist.tile([4, 4], BF16, name="i4b")
        id36b = persist.tile([36, 36], BF16, name="id36b")
        iota16 = persist.tile([128, D], FP32, name="iota16")   # 1..16
        maskRb = persist.tile([128, seq_len, BL], BF16, name="maskRb")
        cmixT = persist.tile([128, KC, seq_len, BL], BF16, name="cmixT")
        buf0 = persist.tile([128, CB, D], FP32, name="buf0")
        buf1 = persist.tile([128, CB, D], FP32, name="buf1")
        h0coll = persist.tile([128, KC, t_out, BL], BF16, name="h0coll")
        if not zero_bias:
            btmT = persist.tile([128, 2, KC], FP32, name="btmT")
            b_out_r = persist.tile([128, C], FP32, name="b_out_r")
            cdecT = persist.tile([128, KC, BL], BF16, name="cdecT")

        # ------------- setup (scoped pools) -------------
        with tc.tile_pool(name="setup_ps", bufs=2, space="PSUM") as setup_ps, \
                tc.tile_pool(name="setup_sb", bufs=1) as setup_sb:
            # raw fp32 weight loads
            wh_d = setup_sb.tile([128, KC, H], FP32, name="wh_d")
            wpass_d = setup_sb.tile([128, KC, H], FP32, name="wpass_d")
            wtau_d = setup_sb.tile([128, KC, H], FP32, name="wtau_d")
            wmem_d = setup_sb.tile([128, KC, H], FP32, name="wmem_d")
            wx_d = setup_sb.tile([128, H], FP32, name="wx_d")
            wout_d = setup_sb.tile([128, KC, C], FP32, name="wout_d")
            nc.sync.dma_start(wh_d[:], dwin[:H].rearrange(
                "(kc kp) n -> kp kc n", kp=128))
            nc.sync.dma_start(wx_d[:], dwin[H:])
            nc.sync.dma_start(wpass_d[:], dwpass[:].rearrange(
                "(kc kp) n -> kp kc n", kp=128))
            nc.sync.dma_start(wtau_d[:], dwtau[:].rearrange(
                "(kc kp) n -> kp kc n", kp=128))
            nc.sync.dma_start(wmem_d[:], dwmem[:].rearrange(
                "(kc kp) n -> kp kc n", kp=128))
            nc.sync.dma_start(wout_d[:], dwout[:].rearrange(
                "(kc kp) n -> kp kc n", kp=128))
            # bf16 copies
            nc.vector.tensor_copy(whb[:], wh_d[:])
            nc.vector.tensor_copy(wtaub[:], wtau_d[:])
            nc.vector.tensor_copy(wmemb[:], wmem_d[:])
            nc.vector.tensor_copy(woutb[:], wout_d[:])
            # fp32r copies for setup GEMMs
            wpass_r = setup_sb.tile([128, KC, H], FP32R, name="wpass_r")
            wx_r = setup_sb.tile([128, H], FP32R, name="wx_r")
            nc.vector.tensor_copy(wpass_r[:], wpass_d[:])
            nc.vector.tensor_copy(wx_r[:], wx_d[:])

            id128 = setup_sb.tile([128, 128], FP32, name="id128")
            make_identity(nc, id128[:])
            i36f = setup_sb.tile([36, 36], FP32, name="i36f")
            make_identity(nc, i36f[:])
            nc.vector.tensor_copy(id36b[:], i36f[:])
            nc.vector.tensor_copy(i4b[:], i36f[0:4, 0:4])

            iota16_i = setup_sb.tile([128, D], I32, name="iota16_i")
            nc.gpsimd.iota(iota16_i[:], pattern=[[1, D]], base=1,
                           channel_multiplier=0)
            nc.vector.tensor_copy(iota16[:], iota16_i[:])

            # masks: maskRb[p, t, b] = t < len[b]
            iota_t = setup_sb.tile([128, seq_len], I32, name="iota_t")
            nc.gpsimd.iota(iota_t[:], pattern=[[1, seq_len]], base=0,
                           channel_multiplier=0)
            lenR = setup_sb.tile([128, BL], I32, name="lenR")
            nc.sync.dma_start(
                lenR[:], dlen[:].unsqueeze(0).to_broadcast([128, BL]))
            mkR_i = setup_sb.tile([128, seq_len, BL], I32, name="mkR_i")
            nc.vector.tensor_tensor(
                out=mkR_i[:],
                in0=iota_t[:].unsqueeze(2).to_broadcast([128, seq_len, BL]),
                in1=lenR[:].unsqueeze(1).to_broadcast([128, seq_len, BL]),
                op=Op.is_lt)
            nc.vector.tensor_copy(maskRb[:], mkR_i[:])

            # mask_bt[p, m]: row-tile masks for the cmix predication
            iota_bt = setup_sb.tile([128, TPB], I32, name="iota_bt")
            nc.gpsimd.iota(iota_bt[:], pattern=[[128, TPB]], base=0,
                           channel_multiplier=1)
            mk_bt_i = setup_sb.tile([128, BL, TPB], I32, name="mk_bt_i")
            nc.vector.tensor_tensor(
                out=mk_bt_i[:],
                in0=iota_bt[:].unsqueeze(1).to_broadcast([128, BL, TPB]),
                in1=lenR[:].unsqueeze(2).to_broadcast([128, BL, TPB]),
                op=Op.is_lt)

            # WhT / WxT via PE transposes (fp32), for Wp2 / Wxp GEMMs
            whT = setup_sb.tile([128, KC, H], FP32R, name="whT")
            wxT = setup_sb.tile([128, KC, I], FP32R, name="wxT")
            for jc in range(KC):
                for kc in range(KC):
                    pst = setup_ps.tile([128, 128], FP32, tag="setup_T")
                    nc.tensor.transpose(pst[:], wh_d[:, kc, bass.ts(jc, 128)],
                                        id128[:])
                    nc.vector.tensor_copy(whT[:, jc, bass.ts(kc, 128)],
                                          pst[:])
            for jc in range(KC):
                pst = setup_ps.tile([128, 128], FP32, tag="setup_T")
                nc.tensor.transpose(pst[:], wx_d[:, bass.ts(jc, 128)],
                                    id128[:])
                nc.vector.tensor_copy(wxT[:, jc, :], pst[:])

            # Wp2 = Wh @ W_pass ; Wxp = Wx @ W_pass  (fp32r GEMMs)
            wp2f = setup_sb.tile([128, KC, H], FP32, name="wp2f")
            wxp = setup_sb.tile([128, H], FP32R, name="wxp")
            for m in range(KC):
                psg = setup_ps.tile([128, H], FP32, tag="setup_G")
                for jc in range(KC):
                    nc.tensor.matmul(psg[:], whT[:, jc, bass.ts(m, 128)],
                                     wpass_r[:, jc, :],
                                     start=(jc == 0), stop=(jc == KC - 1))
                nc.vector.tensor_copy(wp2f[:, m, :], psg[:])
            psg = setup_ps.tile([128, H], FP32, tag="setup_G")
            for jc in range(KC):
                nc.tensor.matmul(psg[:], wxT[:, jc, :], wpass_r[:, jc, :],
                                 start=(jc == 0), stop=(jc == KC - 1))
            nc.vector.tensor_copy(wxp[:], psg[:])
            nc.vector.tensor_copy(wp2b[:], wp2f[:])
            nc.vector.tensor_tensor(out=wdb[:], in0=wp2f[:], in1=wh_d[:],
                                    op=Op.subtract)

            # bias constants
            if not zero_bias:
                btm_d = setup_sb.tile([128, 2, KC], FP32, name="btm_d")
                nc.sync.dma_start(btm_d[:, 0, :], dbias["b_tau"][:].rearrange(
                    "(c p) -> p c", p=128))
                nc.sync.dma_start(btm_d[:, 1, :], dbias["b_mem"][:].rearrange(
                    "(c p) -> p c", p=128))
                nc.vector.tensor_copy(btmT[:], btm_d[:])
                nc.sync.dma_start(
                    b_out_r[:], dbias["b_out"][:].unsqueeze(0)
                    .to_broadcast([128, C]))
                b_in_r = setup_sb.tile([128, H], FP32, name="b_in_r")
                nc.sync.dma_start(
                    b_in_r[:], dbias["b_in"][:].unsqueeze(0)
                    .to_broadcast([128, H]))
                # cdec = b_in @ W_pass + b_pass, in transposed layout
                binT4 = setup_sb.tile([128, KC, BL], FP32R, name="binT4")
                binT4_d = setup_sb.tile([128, KC, BL], FP32, name="binT4_d")
                nc.sync.dma_start(
                    binT4_d[:],
                    dbias["b_in"][:].rearrange("(c p) -> p c", p=128)
                    .unsqueeze(2).to_broadcast([128, KC, BL]))
                nc.vector.tensor_copy(binT4[:], binT4_d[:])
                psd = setup_ps.tile([BL, H], FP32, tag="setup_D")
                for c in range(KC):
                    nc.tensor.matmul(psd[:], binT4[:, c, :], wpass_r[:, c, :],
                                     start=(c == 0), stop=(c == KC - 1))
                bps = setup_sb.tile([BL, H], FP32, name="bps")
                nc.sync.dma_start(
                    bps[:], dbias["b_pass"][:].unsqueeze(0)
                    .to_broadcast([BL, H]))
                cdec4 = setup_sb.tile([BL, H], FP32, name="cdec4")
                nc.vector.tensor_tensor(out=cdec4[:], in0=psd[:],
                                        in1=bps[:], op=Op.add)
                # transpose cdec4 [BL, H] -> cdecT [128, KC, BL]
                for kc in range(KC):
                    pst2 = setup_ps.tile([128, BL], FP32, tag="setup_T2")
                    nc.tensor.transpose(pst2[:], cdec4[:, bass.ts(kc, 128)],
                                        i36f[0:BL, 0:BL])
                    nc.vector.tensor_copy(cdecT[:, kc, :], pst2[:])
                cdec_r = setup_sb.tile([128, H], FP32, name="cdec_r")
                dcdec = nc.dram_tensor("cdec_scratch", [H], FP32)
                nc.sync.dma_start(dcdec[:], cdec4[0:1, :].squeeze(0))
                nc.sync.dma_start(
                    cdec_r[:], dcdec[:].unsqueeze(0).to_broadcast([128, H]))

            # x -> xT ; cx/cp precompute; predicated merge; transpose into
            # SBUF-resident cmixT [128, kc, t, b] (bf16)
            x_sb = setup_sb.tile([128, NMT, I], FP32, name="x_sb")
            xT = setup_sb.tile([128, NMT, 128], FP32R, name="xT")
            nc.sync.dma_start(
                x_sb[:],
                dx[:].rearrange("b t i -> (b t) i").rearrange(
                    "(m p) i -> p m i", p=128))
            for m in range(NMT):
                pst = setup_ps.tile([128, 128], FP32, tag="setup_T")
                nc.tensor.transpose(pst[:], x_sb[:, m, :], id128[:])
                nc.vector.tensor_copy(xT[:, m, :], pst[:])
            for m in range(NMT):
                b_of_m, j_of_m = m // TPB, m % TPB
                ps1 = setup_ps.tile([128, H], FP32, tag="setup_G")
                nc.tensor.matmul(ps1[:], xT[:, m, :], wx_r[:],
                                 start=True, stop=True)
                cxt = setup_sb.tile([128, H], FP32, tag="cxt", bufs=3)
                if zero_bias:
                    nc.vector.tensor_copy(cxt[:], ps1[:])
                else:
                    nc.vector.tensor_tensor(out=cxt[:], in0=ps1[:],
                                            in1=b_in_r[:], op=Op.add)
                ps2 = setup_ps.tile([128, H], FP32, tag="setup_G")
                nc.tensor.matmul(ps2[:], xT[:, m, :], wxp[:],
                                 start=True, stop=True)
                cpt = setup_sb.tile([128, H], FP32, tag="cpt", bufs=3)
                if zero_bias:
                    nc.vector.tensor_copy(cpt[:], ps2[:])
                else:
                    nc.vector.tensor_tensor(out=cpt[:], in0=ps2[:],
                                            in1=cdec_r[:], op=Op.add)
                nc.vector.copy_predicated(
                    out=cxt[:],
                    mask=mk_bt_i[:, b_of_m, j_of_m].unsqueeze(1)
                    .to_broadcast([128, H]),
                    data=cpt[:])
                # transpose merged cmix tile into cmixT
                t_base = j_of_m * 128
                for kc in range(KC):
                    pst = setup_ps.tile([128, 128], FP32, tag="setup_T")
                    nc.tensor.transpose(pst[:], cxt[:, bass.ts(kc, 128)],
                                        id128[:])
                    nc.vector.tensor_copy(
                        cmixT[:, kc, t_base:t_base + 128, b_of_m], pst[:])

        # ------------- main recurrence -------------
        psum = ctx.enter_context(tc.tile_pool(name="mn_ps", bufs=1,
                                              space="PSUM"))
        loop_sb = ctx.enter_context(tc.tile_pool(name="mn_sb", bufs=2))

        bufs = [buf0, buf1]

        g = loop_sb.tile([128, CB], BF16, tag="g")
        nc.vector.memset(f32(g[:]), 0.0)   # [128,16] bf16 as [128,8] fp32
        nc.gpsimd.memset(buf0[:], 0.0)

        # keep-warm junk operands + psum bank (no reader, PE-only traffic)
        junk_s = persist.tile([128, 4], BF16, name="junk_s")
        junk_m = persist.tile([128, 256], BF16, name="junk_m")
        nc.vector.memset(f32(junk_s[:]), 0.0)
        nc.vector.memset(f32(junk_m[:]), 0.0)
        ps_junk = psum.tile([BL, 256], FP32, tag="ps_junk")

        def keep_warm(n):
            for _ in range(n):
                nc.tensor.matmul(ps_junk[:], junk_s[:], junk_m[:],
                                 start=True, stop=True)

        deferred = []

        buf_idx = 0
        total_steps = seq_len + t_out
        for t in range(total_steps):
            is_enc = t < seq_len
            td = t - seq_len
            last = (t == total_steps - 1)
            g_v = g[:].rearrange("p (c b) -> p c b", c=KC)

            if not is_enc:
                nc.vector.tensor_copy(h0coll[:, :, td, :], g_v)
                if last:
                    break

            if is_enc:
                lb = loop_sb.tile([128, KC, BL], BF16, tag="lb")
                nc.vector.tensor_tensor(
                    out=lb[:], in0=g_v,
                    in1=maskRb[:, t, :].unsqueeze(1)
                    .to_broadcast([128, KC, BL]),
                    op=Op.mult)

            # deferred buffer update from the previous step
            while deferred:
                deferred.pop(0)()

            # L1 (row form): ps_h[4,512] = g@Wh + lb@Wd   (or g@Wp2 decode)
            ps_h = psum.tile([BL, H], FP32, tag="ps_h")
            if is_enc:
                for c in range(KC):
                    nc.tensor.matmul(ps_h[:], g_v[:, c, :], whb[:, c, :],
                                     start=(c == 0), stop=False)
                for c in range(KC):
                    nc.tensor.matmul(ps_h[:], lb[:, c, :], wdb[:, c, :],
                                     start=False, stop=(c == KC - 1))
            else:
                for c in range(KC):
                    nc.tensor.matmul(ps_h[:], g_v[:, c, :], wp2b[:, c, :],
                                     start=(c == 0), stop=(c == KC - 1))

            keep_warm(FILL_L1)

            # bridge: h_sb (bf16) <- ps_h, split across DVE and ACT
            h_sb = loop_sb.tile([BL, H], BF16, tag="h_sb")
            nc.vector.tensor_copy(h_sb[:, 0:H // 2], ps_h[:, 0:H // 2])
            nc.scalar.copy(h_sb[:, H // 2:], ps_h[:, H // 2:])

            # T1: h_sb -> hT chunks (bf16 transposes), then add cmixT
            ps_t1 = psum.tile([128, KC, BL], BF16, tag="ps_t1")
            for c in range(KC):
                nc.tensor.transpose(ps_t1[:, c, :], h_sb[:, bass.ts(c, 128)],
                                    i4b[:])
            hT = loop_sb.tile([128, KC, BL], BF16, tag="hT")
            if is_enc:
                nc.vector.tensor_tensor(out=hT[:], in0=ps_t1[:],
                                        in1=cmixT[:, :, t, :], op=Op.add)
            elif zero_bias:
                nc.vector.tensor_copy(hT[:], ps_t1[:])
            else:
                nc.vector.tensor_tensor(out=hT[:], in0=ps_t1[:],
                                        in1=cdecT[:], op=Op.add)

            # L2: tau/mem row GEMMs.  With COL_TILE_L2 the two moving
            # streams run concurrently on separate 32-col strips of the
            # array; mem output then lives at PSUM partitions 32-35 and
            # must stay partition-aligned through its copy + transpose.
            ps_l2 = psum.tile([128, H], FP32, tag="ps_l2")
            for c in range(KC):
                nc.tensor.matmul(ps_l2[0:BL, :], hT[:, c, :],
                                 wtaub[:, c, :], start=(c == 0),
                                 stop=(c == KC - 1),
                                 tile_position=(0, 0) if COL_TILE_L2
                                 else None)
            if COL_TILE_L2:
                for c in range(KC):
                    nc.tensor.matmul(ps_l2[32:32 + BL, :], hT[:, c, :],
                                     wmemb[:, c, :], start=(c == 0),
                                     stop=(c == KC - 1),
                                     tile_position=(0, 32))
            else:
                # without tiling both chains target partitions 0-3; give
                # mem its own bank via a second psum tile
                ps_l2b = psum.tile([BL, H], FP32, tag="ps_l2b")
                for c in range(KC):
                    nc.tensor.matmul(ps_l2b[:], hT[:, c, :],
                                     wmemb[:, c, :], start=(c == 0),
                                     stop=(c == KC - 1))

            keep_warm(FILL_L2)

            # psum -> sbuf (bf16), partition-aligned staging; separate
            # tiles so the two engine copies run concurrently
            ttau = loop_sb.tile([BL, H], BF16, tag="ttau")
            nc.scalar.copy(ttau[:], ps_l2[0:BL, :])
            if COL_TILE_L2:
                tmem = loop_sb.tile([36, H], BF16, tag="tmem")
                nc.vector.tensor_copy(tmem[32:32 + BL, :],
                                      ps_l2[32:32 + BL, :])
                mem_sb, i4m = tmem[32:32 + BL, :], id36b[32:36, 32:36]
            else:
                tmem = loop_sb.tile([BL, H], BF16, tag="tmem")
                nc.vector.tensor_copy(tmem[:], ps_l2b[:])
                mem_sb, i4m = tmem[:], i4b[:]

            # T2: 4+4 transposes -> [128, (tau/mem), kc, b] bf16 psum
            ps_t2 = psum.tile([128, 2, KC, BL], BF16, tag="ps_t2")
            for c in range(KC):
                nc.tensor.transpose(ps_t2[:, 0, c, :],
                                    ttau[:, bass.ts(c, 128)], i4b[:])
            for c in range(KC):
                nc.tensor.transpose(ps_t2[:, 1, c, :],
                                    mem_sb[:, bass.ts(c, 128)], i4m)

            keep_warm(FILL_T2)

            # sigmoid straight from PSUM; tau half first so the DVE tail
            # can start while mem's sigmoid still runs on ACT
            sig = loop_sb.tile([128, 2, KC, BL], BF16, tag="sig")
            if zero_bias:
                nc.scalar.activation(sig[:, 0], ps_t2[:, 0], Sig)
                nc.scalar.activation(sig[:, 1], ps_t2[:, 1], Sig)
            else:
                tm_sb = loop_sb.tile([128, 2, KC, BL], FP32, tag="tm_sb")
                nc.vector.tensor_tensor(
                    out=tm_sb[:], in0=ps_t2[:],
                    in1=btmT[:].unsqueeze(3)
                    .to_broadcast([128, 2, KC, BL]),
                    op=Op.add)
                nc.scalar.activation(sig[:, 0], tm_sb[:, 0], Sig)
                nc.scalar.activation(sig[:, 1], tm_sb[:, 1], Sig)

            # DVE tail: taup, q, 1/taup, t1, g'
            taup = loop_sb.tile([128, CB], FP32, tag="taup")
            nc.vector.tensor_scalar(
                out=taup[:].rearrange("p (c b) -> p c b", c=KC),
                in0=sig[:, 0], scalar1=16.0, scalar2=1.0,
                op0=Op.mult, op1=Op.max)
            q = loop_sb.tile([128, CB], FP32, tag="q")
            nc.vector.tensor_tensor(
                out=q[:].rearrange("p (c b) -> p c b", c=KC),
                in0=sig[:, 1], in1=hT[:], op=Op.mult)
            rtau = loop_sb.tile([128, CB], FP32, tag="rtau")
            rts = loop_sb.tile([128, CB], FP32, tag="rts")
            nc.vector.reciprocal_approx_accurate(
                out=rtau[:], in_=taup[:], scratch=rts[:])
            t1 = loop_sb.tile([128, CB], FP32, tag="t1")
            nc.vector.tensor_tensor(out=t1[:], in0=q[:], in1=rtau[:],
                                    op=Op.mult)
            bcur, bnxt = bufs[buf_idx], bufs[buf_idx ^ 1]
            buf_idx ^= 1
            g = loop_sb.tile([128, CB], BF16, tag="g")
            nc.vector.tensor_tensor(out=g[:], in0=bcur[:, :, 0], in1=t1[:],
                                    op=Op.add)

            # deferred: slots 2..16 shift-add (runs during next step)
            if t < total_steps - 2:
                def make_deferred(taup=taup, q=q, bcur=bcur, bnxt=bnxt):
                    def emit():
                        wt = loop_sb.tile([128, CB, D], FP32, tag="wt")
                        nc.vector.tensor_tensor(
                            out=wt[:],
                            in0=taup[:].unsqueeze(2).to_broadcast(
                                [128, CB, D]),
                            in1=iota16[:].unsqueeze(1).to_broadcast(
                                [128, CB, D]),
                            op=Op.subtract)
                        nc.scalar.activation(wt[:], wt[:], Abs)
                        nc.vector.tensor_scalar(
                            out=wt[:], in0=wt[:], scalar1=1.0, scalar2=None,
                            op0=Op.add)
                        wr = loop_sb.tile([128, CB, D], FP32, tag="wr")
                        ws = loop_sb.tile([128, CB, D], FP32, tag="ws")
                        nc.vector.reciprocal_approx_accurate(
                            out=wr[:], in_=wt[:], scratch=ws[:])
                        nc.vector.tensor_tensor(
                            out=wr[:], in0=wr[:],
                            in1=q[:].unsqueeze(2).to_broadcast([128, CB, D]),
                            op=Op.mult)
                        nc.gpsimd.tensor_tensor(
                            out=bnxt[:, :, 0:D - 1], in0=bcur[:, :, 1:D],
                            in1=wr[:, :, 1:D], op=Op.add)
                        nc.gpsimd.memset(bnxt[:, :, D - 1], 0.0)
                    return emit
                deferred.append(make_deferred())

        # batched output GEMM: out = h0coll @ W_out (+ b_out)
        rows = t_out * BL
        for start in range(0, rows, 128):
            mrows = min(128, rows - start)
            t0, tn = start // BL, mrows // BL
            ps_o = psum.tile([mrows, C], FP32, tag="ps_o")
            for c in range(KC):
                nc.tensor.matmul(
                    ps_o[:],
                    h0coll[:, c, t0:t0 + tn, :].rearrange(
                        "p t b -> p (t b)"),
                    woutb[:, c, :],
                    start=(c == 0), stop=(c == KC - 1))
            out_sb = loop_sb.tile([mrows, C], FP32, tag="out_sb")
            if zero_bias:
                nc.vector.tensor_copy(out_sb[:], ps_o[:])
            else:
                nc.vector.tensor_tensor(out=out_sb[:], in0=ps_o[:],
                                        in1=b_out_r[0:mrows, :], op=Op.add)
            for tt in range(tn):
                nc.sync.dma_start(dout[:, t0 + tt, :],
                                  out_sb[bass.ts(tt, BL), :])

    nc.compile()
    return nc


_CACHE = {}


def _get_module(seq_len, t_out, zero_bias):
    key = (seq_len, t_out, zero_bias)
    if key not in _CACHE:
        _CACHE[key] = build(seq_len, t_out, zero_bias)
    return _CACHE[key]


def kernel(**inputs):
    x = np.ascontiguousarray(np.asarray(inputs["x"], dtype=np.float32))
    lengths = np.ascontiguousarray(
        np.asarray(inputs["lengths"]).astype(np.int32))
    t_out = int(inputs["out_lengths"])
    seq_len = x.shape[1]
    names = ["W_in", "W_pass", "W_tau", "W_mem", "W_out",
             "b_in", "b_pass", "b_tau", "b_mem", "b_out"]
    warrs = {n: np.ascontiguousarray(np.asarray(inputs[n], dtype=np.float32))
             for n in names}
    zero_bias = all(not np.any(warrs[n]) for n in
                    ["b_in", "b_pass", "b_tau", "b_mem", "b_out"])
    nc = _get_module(seq_len, t_out, zero_bias)

    from concourse import bass_utils
    in_maps = []
    for c in range(NCORES):
        sl = slice(c * BL, (c + 1) * BL)
        m = {"x": x[sl], "lengths": lengths[sl]}
        m.update(warrs)
        in_maps.append(m)
    res = bass_utils.run_bass_kernel_spmd(
        nc, in_maps, core_ids=list(range(NCORES)))
    out = np.concatenate([res.results[c]["out"] for c in range(NCORES)],
                         axis=0)
    return out


# revision 3
# speedup vs baseline: 1.1777x; 1.0191x over previous
"""DelayRNN Trainium2 kernel, v2: bf16 row-form recurrence.

Sharding: data-parallel over batch, 4 rows/core on 8 cores (as v1).

Changes vs v1 (fp32r, 4.45 ms):
  * all matmul operands bf16 — moving weight streams run ~5x faster than
    the fp32r ones measured on HW (fp32 LDWEIGHTS + fp32_mode matmuls
    dominated v1), transposes ~3.5x faster
  * cmix (x-dependent injection with the binary length mask folded in)
    lives in SBUF transposed ([128,(kc,t,b)] bf16) and is added during
    the psum->sbuf hop after T1 — no per-step DRAM DMA, no [4,512] add
  * tau/mem copies merge into one [8,512] tile so T2 is 4 transposes
    (not 8); sigmoid reads the T2 PSUM directly (ACT psum port)
  * L2 tau/mem chains are col-tiled to distinct 32-col strips of the PE
    array (tile_position), streaming concurrently on separate XBUSes
  * 1/tau via reciprocal_approx_accurate (2 cheap DVE ops)

Per-step chain: DVE(lb) -> PE(L1 8mm) -> DVE+ACT(bridge copy halves) ->
PE(T1 x4) -> DVE(hT = T1 + cmixT_t) -> PE(L2 2x4mm col-tiled) ->
ACT+DVE(t8 copy halves) -> PE(T2 x4 merged) -> ACT(sigmoid from psum)
-> DVE(taup, q, 1/taup, t1, g'). Buffer slots 2..16 stay deferred one
step and run on DVE/ACT/Pool underneath the next step's PE blocks.
"""

import sys
import numpy as np

for _p in ("/opt/trn_rl_repo",):
    if _p not in sys.path:
        sys.path.append(_p)

from contextlib import ExitStack

import concourse.bass as bass
import concourse.tile as tile
from concourse import bacc, mybir
from concourse.masks import make_identity

FP32 = mybir.dt.float32
FP32R = mybir.dt.float32r
BF16 = mybir.dt.bfloat16
I32 = mybir.dt.int32

B, S, I, H, C = 32, 256, 128, 512, 64
T_OUT = 64
NCORES = 8
BL = B // NCORES        # 4 batch rows per core
KC = H // 128           # 4 k-chunks
D = 16                  # delay slots 1..16
CB = KC * BL            # 16 = chunks x batch

Sig = mybir.ActivationFunctionType.Sigmoid
Abs = mybir.ActivationFunctionType.Abs
Op = mybir.AluOpType

COL_TILE_L2 = True      # run tau/mem moving streams on separate col strips
# keep-warm filler matmuls per PE gap (HAM re-throttles the PE clock to
# 1.2 GHz when it sees idle windows; these keep it at 2.4 GHz)
FILL_L1, FILL_T1, FILL_L2, FILL_T2 = 5, 3, 6, 14


def f32(ap):
    return ap.bitcast(FP32)


def build(seq_len=S, t_out=T_OUT, zero_bias=True):
    nc = bacc.Bacc("TRN2", target_bir_lowering=False, debug=False)

    # ---------------- DRAM I/O ----------------
    dx = nc.dram_tensor("x", [BL, seq_len, I], FP32, kind="ExternalInput")
    dlen = nc.dram_tensor("lengths", [BL], I32, kind="ExternalInput")
    dwin = nc.dram_tensor("W_in", [I + H, H], FP32, kind="ExternalInput")
    dwpass = nc.dram_tensor("W_pass", [H, H], FP32, kind="ExternalInput")
    dwtau = nc.dram_tensor("W_tau", [H, H], FP32, kind="ExternalInput")
    dwmem = nc.dram_tensor("W_mem", [H, H], FP32, kind="ExternalInput")
    dwout = nc.dram_tensor("W_out", [H, C], FP32, kind="ExternalInput")
    dbias = {}
    for nm, ln in [("b_in", H), ("b_pass", H), ("b_tau", H),
                   ("b_mem", H), ("b_out", C)]:
        dbias[nm] = nc.dram_tensor(nm, [ln], FP32, kind="ExternalInput")
    dout = nc.dram_tensor("out", [BL, t_out, C], FP32, kind="ExternalOutput")

    NROW = BL * seq_len            # bt rows
    NMT = NROW // 128              # row tiles for the cx/cp precompute
    TPB = seq_len // 128           # row tiles per batch row

    with tile.TileContext(nc) as tc, ExitStack() as ctx:
        persist = ctx.enter_context(tc.tile_pool(name="persist", bufs=1))

        # ------------- persistent SBUF tensors -------------
        whb = persist.tile([128, KC, H], BF16, name="whb")    # moving [kp,c,n]
        wdb = persist.tile([128, KC, H], BF16, name="wdb")    # Wp2 - Wh
        wp2b = persist.tile([128, KC, H], BF16, name="wp2b")
        wtaub = persist.tile([128, KC, H], BF16, name="wtaub")
        wmemb = persist.tile([128, KC, H], BF16, name="wmemb")
        woutb = persist.tile([128, KC, C], BF16, name="woutb")
        i4b = persist.tile([4, 4], BF16, name="i4b")
        id36b = persist.tile([36, 36], BF16, name="id36b")
        iota16 = persist.tile([128, D], FP32, name="iota16")   # 1..16
        maskRb = persist.tile([128, seq_len, BL], BF16, name="maskRb")
        cmixT = persist.tile([128, KC, seq_len, BL], BF16, name="cmixT")
        buf0 = persist.tile([128, CB, D], FP32, name="buf0")
        buf1 = persist.tile([128, CB, D], FP32, name="buf1")
        h0coll = persist.tile([128, KC, t_out, BL], BF16, name="h0coll")
        if not zero_bias:
            btmT = persist.tile([128, 2, KC], FP32, name="btmT")
            b_out_r = persist.tile([128, C], FP32, name="b_out_r")
            cdecT = persist.tile([128, KC, BL], BF16, name="cdecT")

        # ------------- setup (scoped pools) -------------
        with tc.tile_pool(name="setup_ps", bufs=2, space="PSUM") as setup_ps, \
                tc.tile_pool(name="setup_sb", bufs=1) as setup_sb:
            # raw fp32 weight loads
            wh_d = setup_sb.tile([128, KC, H], FP32, name="wh_d")
            wpass_d = setup_sb.tile([128, KC, H], FP32, name="wpass_d")
            wtau_d = setup_sb.tile([128, KC, H], FP32, name="wtau_d")
            wmem_d = setup_sb.tile([128, KC, H], FP32, name="wmem_d")
            wx_d = setup_sb.tile([128, H], FP32, name="wx_d")
            wout_d = setup_sb.tile([128, KC, C], FP32, name="wout_d")
            nc.sync.dma_start(wh_d[:], dwin[:H].rearrange(
                "(kc kp) n -> kp kc n", kp=128))
            nc.sync.dma_start(wx_d[:], dwin[H:])
            nc.sync.dma_start(wpass_d[:], dwpass[:].rearrange(
                "(kc kp) n -> kp kc n", kp=128))
            nc.sync.dma_start(wtau_d[:], dwtau[:].rearrange(
                "(kc kp) n -> kp kc n", kp=128))
            nc.sync.dma_start(wmem_d[:], dwmem[:].rearrange(
                "(kc kp) n -> kp kc n", kp=128))
            nc.sync.dma_start(wout_d[:], dwout[:].rearrange(
                "(kc kp) n -> kp kc n", kp=128))
            # bf16 copies
            nc.vector.tensor_copy(whb[:], wh_d[:])
            nc.vector.tensor_copy(wtaub[:], wtau_d[:])
            nc.vector.tensor_copy(wmemb[:], wmem_d[:])
            nc.vector.tensor_copy(woutb[:], wout_d[:])
            # fp32r copies for setup GEMMs
            wpass_r = setup_sb.tile([128, KC, H], FP32R, name="wpass_r")
            wx_r = setup_sb.tile([128, H], FP32R, name="wx_r")
            nc.vector.tensor_copy(wpass_r[:], wpass_d[:])
            nc.vector.tensor_copy(wx_r[:], wx_d[:])

            id128 = setup_sb.tile([128, 128], FP32, name="id128")
            make_identity(nc, id128[:])
            i36f = setup_sb.tile([36, 36], FP32, name="i36f")
            make_identity(nc, i36f[:])
            nc.vector.tensor_copy(id36b[:], i36f[:])
            nc.vector.tensor_copy(i4b[:], i36f[0:4, 0:4])

            iota16_i = setup_sb.tile([128, D], I32, name="iota16_i")
            nc.gpsimd.iota(iota16_i[:], pattern=[[1, D]], base=1,
                           channel_multiplier=0)
            nc.vector.tensor_copy(iota16[:], iota16_i[:])

            # masks: maskRb[p, t, b] = t < len[b]
            iota_t = setup_sb.tile([128, seq_len], I32, name="iota_t")
            nc.gpsimd.iota(iota_t[:], pattern=[[1, seq_len]], base=0,
                           channel_multiplier=0)
            lenR = setup_sb.tile([128, BL], I32, name="lenR")
            nc.sync.dma_start(
                lenR[:], dlen[:].unsqueeze(0).to_broadcast([128, BL]))
            mkR_i = setup_sb.tile([128, seq_len, BL], I32, name="mkR_i")
            nc.vector.tensor_tensor(
                out=mkR_i[:],
                in0=iota_t[:].unsqueeze(2).to_broadcast([128, seq_len, BL]),
                in1=lenR[:].unsqueeze(1).to_broadcast([128, seq_len, BL]),
                op=Op.is_lt)
            nc.vector.tensor_copy(maskRb[:], mkR_i[:])

            # mask_bt[p, m]: row-tile masks for the cmix predication
            iota_bt = setup_sb.tile([128, TPB], I32, name="iota_bt")
            nc.gpsimd.iota(iota_bt[:], pattern=[[128, TPB]], base=0,
                           channel_multiplier=1)
            mk_bt_i = setup_sb.tile([128, BL, TPB], I32, name="mk_bt_i")
            nc.vector.tensor_tensor(
                out=mk_bt_i[:],
                in0=iota_bt[:].unsqueeze(1).to_broadcast([128, BL, TPB]),
                in1=lenR[:].unsqueeze(2).to_broadcast([128, BL, TPB]),
                op=Op.is_lt)

            # WhT / WxT via PE transposes (fp32), for Wp2 / Wxp GEMMs
            whT = setup_sb.tile([128, KC, H], FP32R, name="whT")
            wxT = setup_sb.tile([128, KC, I], FP32R, name="wxT")
            for jc in range(KC):
                for kc in range(KC):
                    pst = setup_ps.tile([128, 128], FP32, tag="setup_T")
                    nc.tensor.transpose(pst[:], wh_d[:, kc, bass.ts(jc, 128)],
                                        id128[:])
                    nc.vector.tensor_copy(whT[:, jc, bass.ts(kc, 128)],
                                          pst[:])
            for jc in range(KC):
                pst = setup_ps.tile([128, 128], FP32, tag="setup_T")
                nc.tensor.transpose(pst[:], wx_d[:, bass.ts(jc, 128)],
                                    id128[:])
                nc.vector.tensor_copy(wxT[:, jc, :], pst[:])

            # Wp2 = Wh @ W_pass ; Wxp = Wx @ W_pass  (fp32r GEMMs)
            wp2f = setup_sb.tile([128, KC, H], FP32, name="wp2f")
            wxp = setup_sb.tile([128, H], FP32R, name="wxp")
            for m in range(KC):
                psg = setup_ps.tile([128, H], FP32, tag="setup_G")
                for jc in range(KC):
                    nc.tensor.matmul(psg[:], whT[:, jc, bass.ts(m, 128)],
                                     wpass_r[:, jc, :],
                                     start=(jc == 0), stop=(jc == KC - 1))
                nc.vector.tensor_copy(wp2f[:, m, :], psg[:])
            psg = setup_ps.tile([128, H], FP32, tag="setup_G")
            for jc in range(KC):
                nc.tensor.matmul(psg[:], wxT[:, jc, :], wpass_r[:, jc, :],
                                 start=(jc == 0), stop=(jc == KC - 1))
            nc.vector.tensor_copy(wxp[:], psg[:])
            nc.vector.tensor_copy(wp2b[:], wp2f[:])
            nc.vector.tensor_tensor(out=wdb[:], in0=wp2f[:], in1=wh_d[:],
                                    op=Op.subtract)

            # bias constants
            if not zero_bias:
                btm_d = setup_sb.tile([128, 2, KC], FP32, name="btm_d")
                nc.sync.dma_start(btm_d[:, 0, :], dbias["b_tau"][:].rearrange(
                    "(c p) -> p c", p=128))
                nc.sync.dma_start(btm_d[:, 1, :], dbias["b_mem"][:].rearrange(
                    "(c p) -> p c", p=128))
                nc.vector.tensor_copy(btmT[:], btm_d[:])
                nc.sync.dma_start(
                    b_out_r[:], dbias["b_out"][:].unsqueeze(0)
                    .to_broadcast([128, C]))
                b_in_r = setup_sb.tile([128, H], FP32, name="b_in_r")
                nc.sync.dma_start(
                    b_in_r[:], dbias["b_in"][:].unsqueeze(0)
                    .to_broadcast([128, H]))
                # cdec = b_in @ W_pass + b_pass, in transposed layout
                binT4 = setup_sb.tile([128, KC, BL], FP32R, name="binT4")
                binT4_d = setup_sb.tile([128, KC, BL], FP32, name="binT4_d")
                nc.sync.dma_start(
                    binT4_d[:],
                    dbias["b_in"][:].rearrange("(c p) -> p c", p=128)
                    .unsqueeze(2).to_broadcast([128, KC, BL]))
                nc.vector.tensor_copy(binT4[:], binT4_d[:])
                psd = setup_ps.tile([BL, H], FP32, tag="setup_D")
                for c in range(KC):
                    nc.tensor.matmul(psd[:], binT4[:, c, :], wpass_r[:, c, :],
                                     start=(c == 0), stop=(c == KC - 1))
                bps = setup_sb.tile([BL, H], FP32, name="bps")
                nc.sync.dma_start(
                    bps[:], dbias["b_pass"][:].unsqueeze(0)
                    .to_broadcast([BL, H]))
                cdec4 = setup_sb.tile([BL, H], FP32, name="cdec4")
                nc.vector.tensor_tensor(out=cdec4[:], in0=psd[:],
                                        in1=bps[:], op=Op.add)
                # transpose cdec4 [BL, H] -> cdecT [128, KC, BL]
                for kc in range(KC):
                    pst2 = setup_ps.tile([128, BL], FP32, tag="setup_T2")
                    nc.tensor.transpose(pst2[:], cdec4[:, bass.ts(kc, 128)],
                                        i36f[0:BL, 0:BL])
                    nc.vector.tensor_copy(cdecT[:, kc, :], pst2[:])
                cdec_r = setup_sb.tile([128, H], FP32, name="cdec_r")
                dcdec = nc.dram_tensor("cdec_scratch", [H], FP32)
                nc.sync.dma_start(dcdec[:], cdec4[0:1, :].squeeze(0))
                nc.sync.dma_start(
                    cdec_r[:], dcdec[:].unsqueeze(0).to_broadcast([128, H]))

            # x -> xT ; cx/cp precompute; predicated merge; transpose into
            # SBUF-resident cmixT [128, kc, t, b] (bf16)
            x_sb = setup_sb.tile([128, NMT, I], FP32, name="x_sb")
            xT = setup_sb.tile([128, NMT, 128], FP32R, name="xT")
            nc.sync.dma_start(
                x_sb[:],
                dx[:].rearrange("b t i -> (b t) i").rearrange(
                    "(m p) i -> p m i", p=128))
            for m in range(NMT):
                pst = setup_ps.tile([128, 128], FP32, tag="setup_T")
                nc.tensor.transpose(pst[:], x_sb[:, m, :], id128[:])
                nc.vector.tensor_copy(xT[:, m, :], pst[:])
            for m in range(NMT):
                b_of_m, j_of_m = m // TPB, m % TPB
                ps1 = setup_ps.tile([128, H], FP32, tag="setup_G")
                nc.tensor.matmul(ps1[:], xT[:, m, :], wx_r[:],
                                 start=True, stop=True)
                cxt = setup_sb.tile([128, H], FP32, tag="cxt", bufs=3)
                if zero_bias:
                    nc.vector.tensor_copy(cxt[:], ps1[:])
                else:
                    nc.vector.tensor_tensor(out=cxt[:], in0=ps1[:],
                                            in1=b_in_r[:], op=Op.add)
                ps2 = setup_ps.tile([128, H], FP32, tag="setup_G")
                nc.tensor.matmul(ps2[:], xT[:, m, :], wxp[:],
                                 start=True, stop=True)
                cpt = setup_sb.tile([128, H], FP32, tag="cpt", bufs=3)
                if zero_bias:
                    nc.vector.tensor_copy(cpt[:], ps2[:])
                else:
                    nc.vector.tensor_tensor(out=cpt[:], in0=ps2[:],
                                            in1=cdec_r[:], op=Op.add)
                nc.vector.copy_predicated(
                    out=cxt[:],
                    mask=mk_bt_i[:, b_of_m, j_of_m].unsqueeze(1)
                    .to_broadcast([128, H]),
                    data=cpt[:])
                # transpose merged cmix tile into cmixT
                t_base = j_of_m * 128
                for kc in range(KC):
                    pst = setup_ps.tile([128, 128], FP32, tag="setup_T")
                    nc.tensor.transpose(pst[:], cxt[:, bass.ts(kc, 128)],
                                        id128[:])
                    nc.vector.tensor_copy(
                        cmixT[:, kc, t_base:t_base + 128, b_of_m], pst[:])

        # ------------- main recurrence -------------
        psum = ctx.enter_context(tc.tile_pool(name="mn_ps", bufs=1,
                                              space="PSUM"))
        loop_sb = ctx.enter_context(tc.tile_pool(name="mn_sb", bufs=2))

        bufs = [buf0, buf1]

        g = loop_sb.tile([128, CB], BF16, tag="g")
        nc.vector.memset(f32(g[:]), 0.0)   # [128,16] bf16 as [128,8] fp32
        nc.gpsimd.memset(buf0[:], 0.0)

        # keep-warm junk operands + psum bank (no reader, PE-only traffic)
        junk_s = persist.tile([128, 4], BF16, name="junk_s")
        junk_m = persist.tile([128, 256], BF16, name="junk_m")
        nc.vector.memset(f32(junk_s[:]), 0.0)
        nc.vector.memset(f32(junk_m[:]), 0.0)
        ps_junk = psum.tile([BL, 256], FP32, tag="ps_junk")

        def keep_warm(n, dep):
            # The stationary reads this step's state tile, pinning the
            # fillers into this step's schedule (dependency-free fillers
            # get hoisted to the start of the kernel by the scheduler).
            for _ in range(n):
                nc.tensor.matmul(ps_junk[:], dep[:, 0:4], junk_m[:],
                                 start=True, stop=True)

        deferred = []

        buf_idx = 0
        total_steps = seq_len + t_out
        for t in range(total_steps):
            is_enc = t < seq_len
            td = t - seq_len
            last = (t == total_steps - 1)
            g_v = g[:].rearrange("p (c b) -> p c b", c=KC)

            if not is_enc:
                nc.vector.tensor_copy(h0coll[:, :, td, :], g_v)
                if last:
                    break

            if is_enc:
                lb = loop_sb.tile([128, KC, BL], BF16, tag="lb")
                nc.vector.tensor_tensor(
                    out=lb[:], in0=g_v,
                    in1=maskRb[:, t, :].unsqueeze(1)
                    .to_broadcast([128, KC, BL]),
                    op=Op.mult)

            # deferred buffer update from the previous step
            while deferred:
                deferred.pop(0)()

            # L1 (row form): ps_h[4,512] = g@Wh + lb@Wd   (or g@Wp2 decode)
            ps_h = psum.tile([BL, H], FP32, tag="ps_h")
            if is_enc:
                for c in range(KC):
                    nc.tensor.matmul(ps_h[:], g_v[:, c, :], whb[:, c, :],
                                     start=(c == 0), stop=False)
                for c in range(KC):
                    nc.tensor.matmul(ps_h[:], lb[:, c, :], wdb[:, c, :],
                                     start=False, stop=(c == KC - 1))
            else:
                for c in range(KC):
                    nc.tensor.matmul(ps_h[:], g_v[:, c, :], wp2b[:, c, :],
                                     start=(c == 0), stop=(c == KC - 1))

            keep_warm(FILL_L1, g)

            # bridge: h_sb (bf16) <- ps_h, split across DVE and ACT
            h_sb = loop_sb.tile([BL, H], BF16, tag="h_sb")
            nc.vector.tensor_copy(h_sb[:, 0:H // 2], ps_h[:, 0:H // 2])
            nc.scalar.copy(h_sb[:, H // 2:], ps_h[:, H // 2:])

            # T1: h_sb -> hT chunks (bf16 transposes), then add cmixT
            ps_t1 = psum.tile([128, KC, BL], BF16, tag="ps_t1")
            for c in range(KC):
                nc.tensor.transpose(ps_t1[:, c, :], h_sb[:, bass.ts(c, 128)],
                                    i4b[:])
            hT = loop_sb.tile([128, KC, BL], BF16, tag="hT")
            if is_enc:
                nc.vector.tensor_tensor(out=hT[:], in0=ps_t1[:],
                                        in1=cmixT[:, :, t, :], op=Op.add)
            elif zero_bias:
                nc.vector.tensor_copy(hT[:], ps_t1[:])
            else:
                nc.vector.tensor_tensor(out=hT[:], in0=ps_t1[:],
                                        in1=cdecT[:], op=Op.add)

            keep_warm(FILL_T1, g)

            # L2: tau/mem row GEMMs.  With COL_TILE_L2 the two moving
            # streams run concurrently on separate 32-col strips of the
            # array; mem output then lives at PSUM partitions 32-35 and
            # must stay partition-aligned through its copy + transpose.
            # tau and mem each get their own psum TILE (bank) so the two
            # evacuation copies don't serialize on a shared-tile dep.
            ps_tau = psum.tile([BL, H], FP32, tag="ps_tau")
            for c in range(KC):
                nc.tensor.matmul(ps_tau[:], hT[:, c, :],
                                 wtaub[:, c, :], start=(c == 0),
                                 stop=(c == KC - 1),
                                 tile_position=(0, 0) if COL_TILE_L2
                                 else None)
            if COL_TILE_L2:
                ps_mem = psum.tile([36, H], FP32, tag="ps_mem")
                for c in range(KC):
                    nc.tensor.matmul(ps_mem[32:32 + BL, :], hT[:, c, :],
                                     wmemb[:, c, :], start=(c == 0),
                                     stop=(c == KC - 1),
                                     tile_position=(0, 32))
                mem_ps = ps_mem[32:32 + BL, :]
            else:
                ps_mem = psum.tile([BL, H], FP32, tag="ps_mem")
                for c in range(KC):
                    nc.tensor.matmul(ps_mem[:], hT[:, c, :],
                                     wmemb[:, c, :], start=(c == 0),
                                     stop=(c == KC - 1))
                mem_ps = ps_mem[:]

            keep_warm(FILL_L2, g)

            # psum -> sbuf (bf16), partition-aligned staging; separate
            # tiles so the two engine copies run concurrently
            ttau = loop_sb.tile([BL, H], BF16, tag="ttau")
            nc.scalar.copy(ttau[:], ps_tau[:])
            if COL_TILE_L2:
                tmem = loop_sb.tile([36, H], BF16, tag="tmem")
                nc.vector.tensor_copy(tmem[32:32 + BL, :], mem_ps)
                mem_sb, i4m = tmem[32:32 + BL, :], id36b[32:36, 32:36]
            else:
                tmem = loop_sb.tile([BL, H], BF16, tag="tmem")
                nc.vector.tensor_copy(tmem[:], mem_ps)
                mem_sb, i4m = tmem[:], i4b[:]

            # T2: 4+4 transposes into separate psum tiles so each half's
            # sigmoid can fire as soon as its own transposes finish
            ps_t2t = psum.tile([128, KC, BL], BF16, tag="ps_t2t")
            ps_t2m = psum.tile([128, KC, BL], BF16, tag="ps_t2m")
            for c in range(KC):
                nc.tensor.transpose(ps_t2t[:, c, :],
                                    ttau[:, bass.ts(c, 128)], i4b[:])
            for c in range(KC):
                nc.tensor.transpose(ps_t2m[:, c, :],
                                    mem_sb[:, bass.ts(c, 128)], i4m)

            keep_warm(FILL_T2, g)

            # sigmoid straight from PSUM; tau half first so the DVE tail
            # can start while mem's sigmoid still runs on ACT
            sig = loop_sb.tile([128, 2, KC, BL], BF16, tag="sig")
            if zero_bias:
                nc.scalar.activation(sig[:, 0], ps_t2t[:], Sig)
                nc.scalar.activation(sig[:, 1], ps_t2m[:], Sig)
            else:
                tm_sb = loop_sb.tile([128, 2, KC, BL], FP32, tag="tm_sb")
                nc.vector.tensor_tensor(
                    out=tm_sb[:, 0], in0=ps_t2t[:],
                    in1=btmT[:, 0].unsqueeze(2)
                    .to_broadcast([128, KC, BL]),
                    op=Op.add)
                nc.vector.tensor_tensor(
                    out=tm_sb[:, 1], in0=ps_t2m[:],
                    in1=btmT[:, 1].unsqueeze(2)
                    .to_broadcast([128, KC, BL]),
                    op=Op.add)
                nc.scalar.activation(sig[:, 0], tm_sb[:, 0], Sig)
                nc.scalar.activation(sig[:, 1], tm_sb[:, 1], Sig)

            # DVE tail: taup, q, 1/taup, t1, g'
            taup = loop_sb.tile([128, CB], FP32, tag="taup")
            nc.vector.tensor_scalar(
                out=taup[:].rearrange("p (c b) -> p c b", c=KC),
                in0=sig[:, 0], scalar1=16.0, scalar2=1.0,
                op0=Op.mult, op1=Op.max)
            q = loop_sb.tile([128, CB], FP32, tag="q")
            nc.vector.tensor_tensor(
                out=q[:].rearrange("p (c b) -> p c b", c=KC),
                in0=sig[:, 1], in1=hT[:], op=Op.mult)
            rtau = loop_sb.tile([128, CB], FP32, tag="rtau")
            nc.vector.reciprocal_approx_fast(out=rtau[:], in_=taup[:])
            t1 = loop_sb.tile([128, CB], FP32, tag="t1")
            nc.vector.tensor_tensor(out=t1[:], in0=q[:], in1=rtau[:],
                                    op=Op.mult)
            bcur, bnxt = bufs[buf_idx], bufs[buf_idx ^ 1]
            buf_idx ^= 1
            g = loop_sb.tile([128, CB], BF16, tag="g")
            nc.vector.tensor_tensor(out=g[:], in0=bcur[:, :, 0], in1=t1[:],
                                    op=Op.add)

            # deferred: slots 2..16 shift-add (runs during next step)
            if t < total_steps - 2:
                def make_deferred(taup=taup, q=q, bcur=bcur, bnxt=bnxt):
                    def emit():
                        # wt/wr prep off DVE where possible: Pool TT is
                        # ~620ns here (its TENSOR_SCALAR is pathologically
                        # slow, so the +1 stays a DVE tensor_scalar)
                        wt = loop_sb.tile([128, CB, D], FP32, tag="wt")
                        nc.gpsimd.tensor_tensor(
                            out=wt[:],
                            in0=taup[:].unsqueeze(2).to_broadcast(
                                [128, CB, D]),
                            in1=iota16[:].unsqueeze(1).to_broadcast(
                                [128, CB, D]),
                            op=Op.subtract)
                        nc.scalar.activation(wt[:], wt[:], Abs)
                        nc.vector.tensor_scalar(
                            out=wt[:], in0=wt[:], scalar1=1.0, scalar2=None,
                            op0=Op.add)
                        wr = loop_sb.tile([128, CB, D], FP32, tag="wr")
                        nc.vector.reciprocal_approx_fast(
                            out=wr[:], in_=wt[:])
                        nc.gpsimd.tensor_tensor(
                            out=wr[:], in0=wr[:],
                            in1=q[:].unsqueeze(2).to_broadcast([128, CB, D]),
                            op=Op.mult)
                        nc.gpsimd.tensor_tensor(
                            out=bnxt[:, :, 0:D - 1], in0=bcur[:, :, 1:D],
                            in1=wr[:, :, 1:D], op=Op.add)
                        nc.gpsimd.memset(bnxt[:, :, D - 1], 0.0)
                    return emit
                deferred.append(make_deferred())

        # batched output GEMM: out = h0coll @ W_out (+ b_out)
        rows = t_out * BL
        for start in range(0, rows, 128):
            mrows = min(128, rows - start)
            t0, tn = start // BL, mrows // BL
            ps_o = psum.tile([mrows, C], FP32, tag="ps_o")
            for c in range(KC):
                nc.tensor.matmul(
                    ps_o[:],
                    h0coll[:, c, t0:t0 + tn, :].rearrange(
                        "p t b -> p (t b)"),
                    woutb[:, c, :],
                    start=(c == 0), stop=(c == KC - 1))
            out_sb = loop_sb.tile([mrows, C], FP32, tag="out_sb")
            if zero_bias:
                nc.vector.tensor_copy(out_sb[:], ps_o[:])
            else:
                nc.vector.tensor_tensor(out=out_sb[:], in0=ps_o[:],
                                        in1=b_out_r[0:mrows, :], op=Op.add)
            for tt in range(tn):
                nc.sync.dma_start(dout[:, t0 + tt, :],
                                  out_sb[bass.ts(tt, BL), :])

    nc.compile()
    return nc


_CACHE = {}


def _get_module(seq_len, t_out, zero_bias):
    key = (seq_len, t_out, zero_bias)
    if key not in _CACHE:
        _CACHE[key] = build(seq_len, t_out, zero_bias)
    return _CACHE[key]


def kernel(**inputs):
    x = np.ascontiguousarray(np.asarray(inputs["x"], dtype=np.float32))
    lengths = np.ascontiguousarray(
        np.asarray(inputs["lengths"]).astype(np.int32))
    t_out = int(inputs["out_lengths"])
    seq_len = x.shape[1]
    names = ["W_in", "W_pass", "W_tau", "W_mem", "W_out",
             "b_in", "b_pass", "b_tau", "b_mem", "b_out"]
    warrs = {n: np.ascontiguousarray(np.asarray(inputs[n], dtype=np.float32))
             for n in names}
    zero_bias = all(not np.any(warrs[n]) for n in
                    ["b_in", "b_pass", "b_tau", "b_mem", "b_out"])
    nc = _get_module(seq_len, t_out, zero_bias)

    from concourse import bass_utils
    in_maps = []
    for c in range(NCORES):
        sl = slice(c * BL, (c + 1) * BL)
        m = {"x": x[sl], "lengths": lengths[sl]}
        m.update(warrs)
        in_maps.append(m)
    res = bass_utils.run_bass_kernel_spmd(
        nc, in_maps, core_ids=list(range(NCORES)))
    out = np.concatenate([res.results[c]["out"] for c in range(NCORES)],
                         axis=0)
    return out
